# revision 4
# baseline (speedup 1.0000x reference)
"""EventDrivenODECell Trainium2 kernel.

Math (reference semantics):
  dt = (t_end - t_start)/5
  5 Euler steps: h += dt * (W3 tanh(W2 tanh(W1a h + [bd1 + W1b te(t)])) + bd3)
    where te(t) depends only on the scalar t -> folded on host into a
    per-step bias  b1s = bd1 + W1b @ te(t_s);  dt folded into W3/bd3.
  event: out = h + sigmoid(Wg ef + bg) * (We2 relu(We1h h + We1e ef + be1) + be2)

Device layout: feature-major activations [feat, batch]; batch sharded 8 ways
(8192 rows/core), processed in 16 column-chunks of 512. All matmuls are
[K<=128,M=128]^T @ [K,512] on the PE with PSUM accumulation over K tiles.
Matmul inputs are float32r (full-rate PE mode, ~12-bit mantissa); the h
accumulator is stored f32r (rounding done by the DVE update op), biases and
PSUM stay fp32.
"""

import os
import sys

sys.path.insert(0, "/opt/trn_rl_repo")

import numpy as np

import concourse.bacc as bacc
import concourse.mybir as mybir
import concourse.tile as tile
from concourse.bass_utils import run_bass_kernel_spmd

B = 65536
HID = 256
EVT = 64
TEMB = 32
NUM_STEPS = 5
N_CORES = 8
R = B // N_CORES          # rows per core
CHUNK = 512
N_CHUNKS = R // CHUNK     # 16
GROUP = 8                 # chunks per group (z-pool live window)

MODE = os.environ.get("KMODE", "f32r")   # "f32r" | "f32"

f32 = mybir.dt.float32
f32r = mybir.dt.float32r

_CACHE = {}

# bias-pack column indices
COL_B1S = 0          # 0..4: per-step layer-1 bias
COL_B2 = 5
COL_B3 = 6
COL_BE1 = 7
COL_BE2 = 8
COL_BG = 9
N_BIAS_COLS = 10


def _build(mode):
    wdt = f32r if mode == "f32r" else f32
    nc = bacc.Bacc("TRN2", target_bir_lowering=False, debug=False,
                   num_devices=N_CORES)

    hT_d = nc.dram_tensor("hT", [HID, R], wdt, kind="ExternalInput")
    efT_d = nc.dram_tensor("efT", [EVT, R], wdt, kind="ExternalInput")
    w1_d = nc.dram_tensor("w1", [HID, HID], wdt, kind="ExternalInput")
    w2_d = nc.dram_tensor("w2", [HID, HID], wdt, kind="ExternalInput")
    w3_d = nc.dram_tensor("w3", [HID, HID], wdt, kind="ExternalInput")
    we1h_d = nc.dram_tensor("we1h", [HID, HID], wdt, kind="ExternalInput")
    we1e_d = nc.dram_tensor("we1e", [EVT, HID], wdt, kind="ExternalInput")
    we2_d = nc.dram_tensor("we2", [HID, HID], wdt, kind="ExternalInput")
    wg_d = nc.dram_tensor("wg", [EVT, HID], wdt, kind="ExternalInput")
    biasp_d = nc.dram_tensor("biasp", [HID, N_BIAS_COLS], f32,
                             kind="ExternalInput")
    outT_d = nc.dram_tensor("outT", [HID, R], f32, kind="ExternalOutput")

    Tanh = mybir.ActivationFunctionType.Tanh
    Relu = mybir.ActivationFunctionType.Relu
    Sigmoid = mybir.ActivationFunctionType.Sigmoid
    add = mybir.AluOpType.add
    mult = mybir.AluOpType.mult

    with tile.TileContext(nc) as tc:
        with (
            tc.tile_pool(name="consts", bufs=1) as consts,
            tc.tile_pool(name="h", bufs=1) as h_pool,
            tc.tile_pool(name="z1", bufs=2 * GROUP + 2) as z1_pool,
            tc.tile_pool(name="z2", bufs=2 * GROUP + 2) as z2_pool,
            tc.tile_pool(name="efc", bufs=4) as ef_pool,
            tc.tile_pool(name="stage", bufs=6) as stage_pool,
            tc.tile_pool(name="psum", bufs=8, space="PSUM") as psum_pool,
        ):
            # ---- constants ----
            def load_w(d, name, kparts, kdim=128):
                ts = []
                for k in range(kparts):
                    t = consts.tile([kdim, HID], wdt, tag=f"{name}{k}")
                    nc.sync.dma_start(t[:], d.ap()[k * kdim:(k + 1) * kdim, :])
                    ts.append(t)
                return ts

            w1 = load_w(w1_d, "w1", 2)
            w2 = load_w(w2_d, "w2", 2)
            w3 = load_w(w3_d, "w3", 2)
            we1h = load_w(we1h_d, "we1h", 2)
            we2 = load_w(we2_d, "we2", 2)
            we1e = load_w(we1e_d, "we1e", 1, kdim=EVT)[0]   # [64, 256]
            wg = load_w(wg_d, "wg", 1, kdim=EVT)[0]         # [64, 256]

            biasp = []
            for m in range(2):
                t = consts.tile([128, N_BIAS_COLS], f32, tag=f"biasp{m}")
                nc.sync.dma_start(t[:], biasp_d.ap()[m * 128:(m + 1) * 128, :])
                biasp.append(t)

            def bcol(m, col):
                return biasp[m][:, col:col + 1]

            # ---- persistent h tiles (f32r in fast mode) ----
            h = [[h_pool.tile([128, CHUNK], wdt, tag=f"h{c}_{m}",
                              name=f"h{c}_{m}")
                  for m in range(2)] for c in range(N_CHUNKS)]
            for c in range(N_CHUNKS):
                for m in range(2):
                    nc.sync.dma_start(
                        h[c][m][:],
                        hT_d.ap()[m * 128:(m + 1) * 128,
                                  c * CHUNK:(c + 1) * CHUNK])

            def dense(out_pool, win, x_tiles, kparts, bias_col, act,
                      extra=None, out_dt=wdt):
                """out[m] = act(sum_k win[k][:,m*128:].T @ x_tiles[k] + bias).

                extra: optional (w_tile, x_tile) pair accumulated on top
                (used for the EVT-dim contributions in the event phase).
                Returns list of 2 SBUF tiles [128, CHUNK].
                """
                outs = []
                for m in range(2):
                    ps = psum_pool.tile([128, CHUNK], f32, tag="ps")
                    n_acc = kparts + (1 if extra is not None else 0)
                    i = 0
                    for k in range(kparts):
                        nc.tensor.matmul(
                            ps[:], win[k][:, m * 128:(m + 1) * 128],
                            x_tiles[k][:],
                            start=(i == 0), stop=(i == n_acc - 1))
                        i += 1
                    if extra is not None:
                        ew, ex = extra
                        nc.tensor.matmul(
                            ps[:], ew[:, m * 128:(m + 1) * 128], ex,
                            start=False, stop=True)
                    o = out_pool.tile([128, CHUNK], out_dt, tag="z")
                    nc.scalar.activation(o[:], ps[:], act,
                                         bias=bcol(m, bias_col))
                    outs.append(o)
                return outs

            groups = [range(g * GROUP, (g + 1) * GROUP)
                      for g in range(N_CHUNKS // GROUP)]

            for chunks in groups:
                # ---- ODE: 5 Euler steps, layer-major within the group ----
                for s in range(NUM_STEPS):
                    z1s = {}
                    z2s = {}
                    for c in chunks:
                        z1s[c] = dense(z1_pool, w1, h[c], 2, COL_B1S + s, Tanh)
                    for c in chunks:
                        z2s[c] = dense(z2_pool, w2, z1s[c], 2, COL_B2, Tanh)
                    for c in chunks:
                        for m in range(2):
                            ps = psum_pool.tile([128, CHUNK], f32, tag="ps")
                            nc.tensor.matmul(ps[:],
                                             w3[0][:, m * 128:(m + 1) * 128],
                                             z2s[c][0][:], start=True,
                                             stop=False)
                            nc.tensor.matmul(ps[:],
                                             w3[1][:, m * 128:(m + 1) * 128],
                                             z2s[c][1][:], start=False,
                                             stop=True)
                            # h += (psum + b3)   (rounds h to wdt on store)
                            nc.vector.scalar_tensor_tensor(
                                h[c][m][:], ps[:], bcol(m, COL_B3),
                                h[c][m][:], op0=add, op1=add)

                # ---- event update for this group ----
                for c in chunks:
                    efc = ef_pool.tile([EVT, CHUNK], wdt, tag="ef")
                    nc.sync.dma_start(
                        efc[:], efT_d.ap()[:, c * CHUNK:(c + 1) * CHUNK])
                    u1 = dense(z1_pool, we1h, h[c], 2, COL_BE1, Relu,
                               extra=(we1e, efc[:]))
                    for m in range(2):
                        psu = psum_pool.tile([128, CHUNK], f32, tag="ps")
                        nc.tensor.matmul(psu[:],
                                         we2[0][:, m * 128:(m + 1) * 128],
                                         u1[0][:], start=True, stop=False)
                        nc.tensor.matmul(psu[:],
                                         we2[1][:, m * 128:(m + 1) * 128],
                                         u1[1][:], start=False, stop=True)
                        psg = psum_pool.tile([128, CHUNK], f32, tag="ps")
                        nc.tensor.matmul(psg[:],
                                         wg[:, m * 128:(m + 1) * 128],
                                         efc[:], start=True, stop=True)
                        gate = z2_pool.tile([128, CHUNK], f32, tag="z")
                        nc.scalar.activation(gate[:], psg[:], Sigmoid,
                                             bias=bcol(m, COL_BG))
                        # tmp = (psum_upd + be2) * gate
                        tmp = z2_pool.tile([128, CHUNK], f32, tag="z")
                        nc.vector.scalar_tensor_tensor(
                            tmp[:], psu[:], bcol(m, COL_BE2), gate[:],
                            op0=add, op1=mult)
                        # out = tmp + h
                        stg = stage_pool.tile([128, CHUNK], f32, tag="st")
                        nc.vector.tensor_add(stg[:], tmp[:], h[c][m][:])
                        nc.sync.dma_start(
                            outT_d.ap()[m * 128:(m + 1) * 128,
                                        c * CHUNK:(c + 1) * CHUNK],
                            stg[:])

    nc.finalize()
    return nc


def _get_nc(mode):
    if mode not in _CACHE:
        _CACHE[mode] = _build(mode)
    return _CACHE[mode]


LAST_RESULT = None


def kernel(h_prev, event_features, t_start, t_end,
           Wt1, bt1, Wt2, bt2,
           Wd1, bd1, Wd2, bd2, Wd3, bd3,
           We1, be1, We2, be2, Wg, bg):
    global LAST_RESULT
    assert h_prev.shape == (B, HID) and event_features.shape == (B, EVT)

    # ---- host-side folding (float64 for exactness, cast to f32) ----
    f8 = np.float64
    dt = (f8(t_end) - f8(t_start)) / NUM_STEPS
    b1s = np.empty((HID, NUM_STEPS), dtype=f8)
    for s in range(NUM_STEPS):
        t = f8(t_start) + s * dt
        te = np.tanh(t * Wt1[:, 0].astype(f8) + bt1.astype(f8))
        te = Wt2.astype(f8) @ te + bt2.astype(f8)
        b1s[:, s] = bd1.astype(f8) + Wd1[:, HID:].astype(f8) @ te

    w1T = np.ascontiguousarray(Wd1[:, :HID].T, dtype=np.float32)
    w2T = np.ascontiguousarray(Wd2.T, dtype=np.float32)
    w3T = np.ascontiguousarray((dt * Wd3.astype(f8)).T.astype(np.float32))
    we1hT = np.ascontiguousarray(We1[:, :HID].T, dtype=np.float32)
    we1eT = np.ascontiguousarray(We1[:, HID:].T, dtype=np.float32)
    we2T = np.ascontiguousarray(We2.T, dtype=np.float32)
    wgT = np.ascontiguousarray(Wg.T, dtype=np.float32)

    biasp = np.zeros((HID, N_BIAS_COLS), dtype=f8)
    biasp[:, COL_B1S:COL_B1S + NUM_STEPS] = b1s
    biasp[:, COL_B2] = bd2.astype(f8)
    biasp[:, COL_B3] = dt * bd3.astype(f8)
    biasp[:, COL_BE1] = be1.astype(f8)
    biasp[:, COL_BE2] = be2.astype(f8)
    biasp[:, COL_BG] = bg.astype(f8)
    biasp = biasp.astype(np.float32)

    hT = np.ascontiguousarray(h_prev.T, dtype=np.float32)      # [HID, B]
    efT = np.ascontiguousarray(event_features.T, dtype=np.float32)

    shared = dict(w1=w1T, w2=w2T, w3=w3T, we1h=we1hT, we1e=we1eT,
                  we2=we2T, wg=wgT, biasp=biasp)
    in_maps = []
    for c in range(N_CORES):
        sl = slice(c * R, (c + 1) * R)
        in_maps.append(dict(
            hT=np.ascontiguousarray(hT[:, sl]),
            efT=np.ascontiguousarray(efT[:, sl]),
            **shared))

    nc = _get_nc(MODE)
    res = run_bass_kernel_spmd(nc, in_maps, core_ids=list(range(N_CORES)))
    LAST_RESULT = res

    out = np.empty((B, HID), dtype=np.float32)
    for c in range(N_CORES):
        out[c * R:(c + 1) * R, :] = res.results[c]["outT"].T
    return out
